# revision 11
# baseline (speedup 1.0000x reference)
"""CapsuleLayer (dynamic routing, 3 iterations) on 8 Trainium2 NeuronCores.

Sharding: hybrid 2 n-halves x 4 b-quarters.  Cores in a pair hold
complementary halves of the input capsules for the same batch quarter, so
the two routing AllReduces run over 2-rank groups.  Payload is
s = [16, 1024] f32.

Pipeline per core (partitions = (n8, b16); free = (c, j) c-major, bf16):
  - Build: u_hat = x*W via block-diagonal-stationary matmuls (K = 8 n's x
    16 i); a dense x pack accumulates s0 = sum_n u_hat in the same pass.
    W streams as bf16 4-group (1MB) chunks; PSUM evacuates to SBUF
    alternating ScalarE/VectorE.
  - Routing sweeps, 3 groups per step: t1 = u*v as a dense 2x-mode
    bf16 tensor_tensor against a broadcast view of v (no materialized
    replica); the j-reduce runs as a 5-level pairwise TT-add tree
    (~1.9us vs 3.26us for tensor_reduce); softmax: small exp on ACT,
    z-reduce + reciprocal on DVE; the normalization 1/z rides the
    per-group matmul stationary (rd_g = delta * r_g) so t2 is a dense
    2x TT against an ACT-expanded exp(lg) field.  A tunable fraction of
    t1 multiplies runs on the Pool engine for balance.
  - The third routing iteration's reduce + squash happen on host: cores
    ship raw per-half partials s2 = [16, 1024].
"""

import numpy as np
from contextlib import ExitStack

import ml_dtypes

import concourse.bass as bass
import concourse.mybir as mybir
from concourse import tile
from concourse.bass_utils import run_bass_kernel_spmd
from concourse.vector_clock import ScopedClock

# Problem constants
B, N, Di = 64, 1152, 16
C, Dc = 32, 32
NCORES = 8
G2 = 2                       # n-halves
K4 = 4                       # b-quarters
NLOC = N // G2               # 576 input capsules per core
NG = NLOC // 8               # 72 groups of 8 n's
BLOC = B // K4               # 16 samples per core
EPS = 1e-7

WCH = 3                      # w-groups per DMA chunk
XCH = 4                      # x-groups per DMA chunk

F32 = mybir.dt.float32
BF16 = mybir.dt.bfloat16


class PatchedTC(tile.TileContext):
    """This walrus build only supports ONE sync-wait per instruction; Tile's
    final drain carries one wait per outstanding DMA-queue semaphore.  Split
    the extras onto single-wait SP nops."""

    def _drain_and_barrier(self, tick_clock, wait_clock):
        nc = self.nc
        drain_inst = nc.sync.drain()
        wait_clock.add_sem_waits(
            drain_inst.ins, ScopedClock({None: tick_clock.global_clock})
        )
        si = drain_inst.ins.sync_info
        if si is not None and len(si.on_wait) > 1:
            waits = list(si.on_wait)
            del si.on_wait[1:]
            for w in waits[1:]:
                n2 = nc.sync.nop()
                if n2.ins.sync_info is None:
                    n2.ins.sync_info = mybir.SyncInfo(on_update=[], on_wait=[w])
                else:
                    n2.ins.sync_info.on_wait.append(w)
        nc.all_engine_barrier()
        popped = nc._tile_sem_poison_stack.pop()
        assert popped is self._sem_poison
        nc.clear_and_free_semaphores(list(self.sems.allocated().values()))
        nc.all_engine_barrier()


def _split_multi_waits(nc):
    """Post-pass: any instruction carrying >1 sync wait gets the extras moved
    onto same-engine nop instructions inserted right before it."""
    for fn in nc.m.functions:
        for bb in fn.blocks:
            insts = list(bb.instructions)
            out = []
            for ins in insts:
                si = getattr(ins, "sync_info", None)
                if si is not None and si.on_wait is not None and len(si.on_wait) > 1:
                    waits = list(si.on_wait)
                    del si.on_wait[1:]
                    for k, w in enumerate(waits[1:]):
                        nop = mybir.InstNoOp(
                            name=f"{ins.name}-wsplit{k}", ins=[], outs=[]
                        )
                        nop.engine = ins.engine
                        nop.sync_info = mybir.SyncInfo(on_update=[], on_wait=[w])
                        out.append(nop)
                out.append(ins)
            if len(out) != len(insts):
                bb.instructions[:] = out


def _view_cj(ap, c=32, j=32):
    """[P, c*j] AP (c-major) -> [P, c, j]."""
    return ap.rearrange("p (c j) -> p c j", c=c, j=j)


def _rep_outer(ap, n):
    """[P, F] AP -> [P, n(step 0), F] broadcast view."""
    lst = [list(p) for p in ap.ap]
    new = [lst[0], [0, n]] + lst[1:]
    return bass.AP(ap.tensor, ap.offset, new)


def _bcast_inner(ap, n):
    """[P, C] AP -> [P, C, n(step 0)] broadcast view."""
    lst = [list(p) for p in ap.ap]
    return bass.AP(ap.tensor, ap.offset, lst + [[0, n]])


GB = 3                       # groups per sweep step
NB = NG // GB                # 24 sweep steps


def build_program(repeat=1, no_ar=False,
                  t1_pool_num=5, t1_pool_den=7,
                  n_sweeps=2, do_build=True, empty=False,
                  evac_dve_num=1, evac_dve_den=2,
                  t2_mode="rd", tree=True,
                  t2_pool_num=0, t2_pool_den=7,
                  vb_one_dma=True):
    nc = bass.Bass()

    w_pack = nc.declare_dram_parameter(
        "w_pack", [NG // WCH, 128, WCH * 1024], BF16, isOutput=False)
    x_pack = nc.declare_dram_parameter(
        "x_pack", [NG // XCH, 128, XCH * 144], BF16, isOutput=False)
    delta = nc.declare_dram_parameter("delta", [128, 16], BF16, isOutput=False)
    out_ext = nc.declare_dram_parameter("out", [BLOC, 1024], F32, isOutput=True)

    ctx = ExitStack()
    with PatchedTC(nc) as tc, ctx:
        sb = ctx.enter_context(tc.tile_pool(name="sb", bufs=1))
        # "stream" serves double duty: w-chunk staging during build (WCH
        # groups = [128, 3072] bf16) and the expanded exp field during the
        # sweeps (same shape) — phases are disjoint, so one 3-deep ring
        # covers both.
        stream = ctx.enter_context(tc.tile_pool(name="stream", bufs=3))
        xpool = ctx.enter_context(tc.tile_pool(name="x", bufs=2))
        psum_u = ctx.enter_context(tc.tile_pool(name="psu", bufs=3, space="PSUM"))
        psum_s = ctx.enter_context(tc.tile_pool(name="pss", bufs=1, space="PSUM"))
        bigpool = ctx.enter_context(tc.tile_pool(name="big", bufs=3))
        trpool = ctx.enter_context(tc.tile_pool(name="tr", bufs=1))
        smpool = ctx.enter_context(tc.tile_pool(name="sm", bufs=4))
        epool = ctx.enter_context(tc.tile_pool(name="e", bufs=4))
        rdpool = ctx.enter_context(tc.tile_pool(name="rd", bufs=12))
        dram = ctx.enter_context(tc.tile_pool(name="dram", bufs=1, space="DRAM"))

        # Persistent SBUF (u_hat split: finer dependency granularity; part
        # size is a multiple of the sweep-step width so steps never straddle)
        n_uparts = 4
        u_parts = [
            sb.tile([128, (NG // n_uparts) * 1024], BF16, tag=f"uhat{i}",
                    name=f"u_sb{i}")
            for i in range(n_uparts)
        ]
        b1_sb = sb.tile([128, NG * 32], BF16, tag="b1")          # 4.5 KB/part
        delta_sb = sb.tile([128, 16], BF16, tag="delta")
        vb_sb = sb.tile([128, 1024], BF16, tag="vbcast")
        s_sb = sb.tile([BLOC, 1024], F32, tag="sfull")
        sq_sb = sb.tile([BLOC, 1024], BF16, tag="sq")
        n2_sb = sb.tile([BLOC, 96], F32, tag="n2")

        nc.sync.dma_start(out=delta_sb[:], in_=delta[:])
        if not do_build and not empty:
            for p in u_parts:
                nc.vector.memset(p[:], 0.01)

        def u_slice(g, ng=1):
            psz = NG // n_uparts
            part, off = g // psz, g % psz
            return u_parts[part][:, off * 1024:(off + ng) * 1024]

        # ---------- Phase 1: u_hat build + s0 accumulation ----------
        def build_uhat():
            ps_s0 = psum_s.tile([BLOC, 1024], F32, tag="s")
            x_t = None
            for ch in range(NG // WCH):
                w_t = stream.tile([128, WCH * 1024], BF16, tag="s3k")
                nc.sync.dma_start(out=w_t[:], in_=w_pack[ch])
                for gi in range(WCH):
                    g = ch * WCH + gi
                    if g % XCH == 0:
                        x_t = xpool.tile([128, XCH * 144], BF16, tag="x")
                        nc.sync.dma_start(out=x_t[:], in_=x_pack[g // XCH])
                    xo = (g % XCH) * 144
                    ps_u = psum_u.tile([128, 1024], F32, tag="u")
                    for h in range(2):
                        sl = slice(gi * 1024 + h * 512, gi * 1024 + (h + 1) * 512)
                        osl = slice(h * 512, (h + 1) * 512)
                        nc.tensor.matmul(
                            ps_u[:, osl], x_t[:, xo:xo + 128], w_t[:, sl],
                            start=True, stop=True,
                        )
                        nc.tensor.matmul(
                            ps_s0[:, osl], x_t[:, xo + 128:xo + 144], w_t[:, sl],
                            start=(g == 0), stop=(g == NG - 1),
                        )
                    usl = u_slice(g)
                    if (g * evac_dve_num) % evac_dve_den < evac_dve_num:
                        nc.vector.tensor_copy(usl[:], ps_u[:])
                    else:
                        nc.scalar.copy(usl[:], ps_u[:])
            return ps_s0

        # ---------- AllReduce + squash ----------
        def allreduce_squash(ps_s, scale0):
            bounce_in = dram.tile([BLOC, 1024], F32, tag="cin")
            bounce_out = dram.tile([BLOC, 1024], F32, tag="cout")
            nc.vector.tensor_scalar(
                s_sb[:], ps_s[:], scale0, None, mybir.AluOpType.mult
            )
            nc.sync.dma_start(out=bounce_in[:], in_=s_sb[:])
            if no_ar:
                nc.sync.dma_start(out=bounce_out[:], in_=bounce_in[:])
            else:
                nc.gpsimd.collective_compute(
                    "AllReduce",
                    mybir.AluOpType.add,
                    replica_groups=[[0, 1], [2, 3], [4, 5], [6, 7]],
                    ins=[bounce_in[:]],
                    outs=[bounce_out[:]],
                )
            nc.sync.dma_start(out=s_sb[:], in_=bounce_out[:])
            # squash: n2 = sum_j s^2 ; vb = bf16(s * n2/(1+n2)/sqrt(n2+eps))
            with nc.allow_low_precision(reason="bf16 squares"):
                nc.vector.tensor_mul(sq_sb[:], s_sb[:], s_sb[:])
            n2 = n2_sb[:, 0:32]
            nc.vector.tensor_reduce(
                n2, _view_cj(sq_sb[:]), mybir.AxisListType.X,
                mybir.AluOpType.add,
            )
            rt = n2_sb[:, 32:64]
            nc.vector.tensor_scalar(rt, n2, EPS, None, mybir.AluOpType.add)
            nc.scalar.activation(rt, rt, mybir.ActivationFunctionType.Sqrt)
            on2 = n2_sb[:, 64:96]
            nc.vector.tensor_scalar(on2, n2, 1.0, None, mybir.AluOpType.add)
            nc.vector.tensor_mul(rt, rt, on2)
            nc.vector.reciprocal(rt, rt)
            nc.vector.tensor_mul(n2, n2, rt)   # n2 <- scale factor
            nc.vector.tensor_tensor(
                _view_cj(vb_sb[0:16, :]), _view_cj(s_sb[:]),
                _bcast_inner(n2, 32), mybir.AluOpType.mult,
            )
            # broadcast over the remaining 7 n8 partition blocks
            if vb_one_dma:
                for n8 in (1, 2, 4):
                    nc.sync.dma_start(
                        out=vb_sb[n8 * 16:2 * n8 * 16, :],
                        in_=vb_sb[0:n8 * 16, :],
                    )
            else:
                for n8 in range(1, 8):
                    nc.sync.dma_start(
                        out=vb_sb[n8 * 16:(n8 + 1) * 16, :], in_=vb_sb[0:16, :],
                    )

        # ---------- Sweep ----------
        def u3_slice(k):
            return u_slice(k * GB, GB)

        def t1_eng(k):
            return (
                nc.gpsimd if (k * t1_pool_num) % t1_pool_den < t1_pool_num
                else nc.vector
            )

        def t2_eng(k):
            return (
                nc.gpsimd if (k * t2_pool_num) % t2_pool_den < t2_pool_num
                else nc.vector
            )

        def bstage1(k, is_b):
            """t1 = u*v (dense 2x TT against broadcast view), tree j-reduce,
            softmax pieces.  Returns what bstage2 needs."""
            t1 = bigpool.tile([128, GB * 1024], BF16, tag="big")
            t1_e = t1_eng(k)
            t1_e.tensor_tensor(
                t1[:].rearrange("p (g f) -> p g f", g=GB),
                u3_slice(k).rearrange("p (g f) -> p g f", g=GB),
                _rep_outer(vb_sb[:], GB),
                mybir.AluOpType.mult,
            )
            # pairwise-add tree over j: 3072 -> 96
            trA = trpool.tile([128, GB * 512], BF16, tag="trA")
            trB = trpool.tile([128, GB * 256], BF16, tag="trB")
            with nc.allow_low_precision(reason="bf16 logits tree"):
                def lvl(src, dst, j):
                    iv = src.rearrange("p (x j) -> p x j", j=j)
                    ov = dst.rearrange("p (x j) -> p x j", j=j // 2)
                    nc.vector.tensor_tensor(
                        ov, iv[:, :, 0:j // 2], iv[:, :, j // 2:j],
                        mybir.AluOpType.add,
                    )
                lvl(t1[:], trA[:], 32)                     # 3072 -> 1536
                lvl(trA[:], trB[:, 0:GB * 256], 16)        # 1536 -> 768
                lvl(trB[:, 0:GB * 256], trA[:, 0:GB * 128], 8)
                lvl(trA[:, 0:GB * 128], trB[:, 0:GB * 64], 4)
                lg_sl = b1_sb[:, k * GB * 32:(k + 1) * GB * 32]
                if not is_b:
                    lvl(trB[:, 0:GB * 64], lg_sl, 2)       # -> b1 store
                    lg = lg_sl
                else:
                    b2 = smpool.tile([128, GB * 32], F32, tag="b2")
                    lvl(trB[:, 0:GB * 64], b2[:], 2)
                    nc.vector.tensor_add(b2[:], b2[:], lg_sl)
                    lg = b2[:]
            e3 = epool.tile([128, GB * 32], BF16, tag="e3")
            nc.scalar.activation(
                e3[:], lg, mybir.ActivationFunctionType.Exp,
            )
            z3 = smpool.tile([128, GB], F32, tag="z3")
            nc.vector.tensor_reduce(
                z3[:], e3[:].rearrange("p (g c) -> p g c", g=GB),
                mybir.AxisListType.X, mybir.AluOpType.add,
            )
            r3 = smpool.tile([128, GB], F32, tag="r3")
            nc.vector.reciprocal(r3[:], z3[:])
            if t2_mode == "rd":
                # expanded unnormalized exp field; 1/z rides the stationary
                e3x = stream.tile([128, GB * 1024], BF16, tag="s3k")
                nc.scalar.activation(
                    e3x[:].rearrange("p (x j) -> p x j", j=32),
                    _bcast_inner(lg, 32),
                    mybir.ActivationFunctionType.Exp,
                )
                rds = []
                for g in range(GB):
                    rd_g = rdpool.tile([128, 16], BF16, tag="rd")
                    nc.vector.tensor_scalar(
                        rd_g[:], delta_sb[:], r3[:, g:g + 1], None,
                        mybir.AluOpType.mult,
                    )
                    rds.append(rd_g)
                return t1, e3x, rds
            else:
                c3 = epool.tile([128, GB * 32], BF16, tag="c3")
                nc.vector.tensor_tensor(
                    c3[:].rearrange("p (g c) -> p g c", g=GB),
                    e3[:].rearrange("p (g c) -> p g c", g=GB),
                    _bcast_inner(r3[:], 32),
                    mybir.AluOpType.mult,
                )
                return t1, c3, None

        def bstage2(k, ps_s, h1):
            t1, mult, rds = h1
            # t2 reuses t1's tile: t1 is dead after tree level 1, and the
            # ring slot stays live until the matmuls below drain it.
            t2 = t1
            t2_e = t2_eng(k)
            if rds is not None:
                t2_e.tensor_tensor(
                    t2[:], u3_slice(k), mult[:], mybir.AluOpType.mult
                )
                stats = [r[:] for r in rds]
            else:
                t2_e.tensor_tensor(
                    t2[:].rearrange("p (x j) -> p x j", j=32),
                    u3_slice(k).rearrange("p (x j) -> p x j", j=32),
                    _bcast_inner(mult[:], 32),
                    mybir.AluOpType.mult,
                )
                stats = [delta_sb[:]] * GB
            for g in range(GB):
                for h in range(2):
                    sl = slice(g * 1024 + h * 512, g * 1024 + (h + 1) * 512)
                    nc.tensor.matmul(
                        ps_s[:, h * 512:(h + 1) * 512], stats[g], t2[:, sl],
                        start=(k == 0 and g == 0),
                        stop=(k == NB - 1 and g == GB - 1),
                    )

        LOOKAHEAD = 2

        def sweep(is_b):
            ps_s = psum_s.tile([BLOC, 1024], F32, tag="s")
            pend = {}
            for k in range(NB + LOOKAHEAD):
                if k < NB:
                    pend[k] = bstage1(k, is_b)
                if k - LOOKAHEAD in pend:
                    bstage2(k - LOOKAHEAD, ps_s, pend.pop(k - LOOKAHEAD))
            return ps_s

        # ---------- Routing ----------
        for _rep in range(repeat):
            if empty:
                nc.vector.tensor_scalar(
                    s_sb[:], s_sb[:], 1.0, None, mybir.AluOpType.mult
                )
                nc.sync.dma_start(out=out_ext[:], in_=s_sb[:])
                continue
            if do_build:
                ps = build_uhat()
            else:
                ps = psum_s.tile([BLOC, 1024], F32, tag="s")
                nc.vector.tensor_scalar(
                    ps[:], s_sb[:], 1.0, None, mybir.AluOpType.mult
                )
            if n_sweeps >= 1:
                allreduce_squash(ps, 1.0 / C)
                ps = sweep(is_b=False)
            if n_sweeps >= 2:
                allreduce_squash(ps, 1.0)
                ps = sweep(is_b=True)
            nc.vector.tensor_copy(s_sb[:], ps[:])
            nc.sync.dma_start(out=out_ext[:], in_=s_sb[:])

    _split_multi_waits(nc)
    return nc


def host_prep(inputs, W, core):
    q4, g2 = core // 2, core % 2
    n0 = g2 * NLOC
    b0 = q4 * BLOC
    Wk = W[:, n0:n0 + NLOC]                                # [C, 576, Dc, Di]
    xk = inputs[b0:b0 + BLOC, n0:n0 + NLOC]                # [16, 576, Di]

    # w_pack[g, n8*16+i, c*32+j] = W[c, n0+g*8+n8, j, i]   (c-major free)
    wg = Wk.reshape(C, NG, 8, Dc, Di)                      # c g n8 j i
    w_pack = np.ascontiguousarray(
        wg.transpose(1, 2, 4, 0, 3).reshape(NG, 128, 1024)
    ).astype(ml_dtypes.bfloat16)
    # chunk: [NG/WCH, 128, WCH*1024] contiguous per (chunk, partition)
    w_pack = np.ascontiguousarray(
        w_pack.reshape(NG // WCH, WCH, 128, 1024).transpose(0, 2, 1, 3)
        .reshape(NG // WCH, 128, WCH * 1024)
    )

    # x arranged [g, n8, i, b]
    xg = xk.reshape(BLOC, NG, 8, Di).transpose(1, 2, 3, 0)  # g n8 i b
    x_pack = np.zeros((NG, 128, 144), dtype=np.float32)
    for n8 in range(8):
        x_pack[:, n8 * 16:(n8 + 1) * 16, n8 * 16:(n8 + 1) * 16] = xg[:, n8]
    x_pack[:, :, 128:144] = xg.reshape(NG, 128, 16)
    x_pack = x_pack.astype(ml_dtypes.bfloat16)
    x_pack = np.ascontiguousarray(
        x_pack.reshape(NG // XCH, XCH, 128, 144).transpose(0, 2, 1, 3)
        .reshape(NG // XCH, 128, XCH * 144)
    )

    delta_np = np.zeros((128, 16), dtype=np.float32)
    for n8 in range(8):
        for b16 in range(16):
            delta_np[n8 * 16 + b16, b16] = 1.0
    delta_np = delta_np.astype(ml_dtypes.bfloat16)

    return {"w_pack": w_pack, "x_pack": x_pack, "delta": delta_np}


def postprocess(partials):
    """partials[core] = s2 partial [16, 1024] (c-major).  Sum n-half pairs,
    squash, assemble [B, C, Dc]."""
    out = np.zeros((B, C, Dc), dtype=np.float32)
    for q4 in range(K4):
        s = (np.asarray(partials[q4 * 2], np.float32)
             + np.asarray(partials[q4 * 2 + 1], np.float32))
        s = s.reshape(BLOC, C, Dc)
        n2 = np.sum(s * s, axis=-1, keepdims=True)
        out[q4 * BLOC:(q4 + 1) * BLOC] = s * (
            n2 / (1.0 + n2) / np.sqrt(n2 + EPS)
        )
    return out


_NC_CACHE = {}


def _get_nc():
    if "nc" not in _NC_CACHE:
        _NC_CACHE["nc"] = build_program()
    return _NC_CACHE["nc"]


def kernel(inputs, W, _trace=False):
    inputs = np.asarray(inputs, dtype=np.float32)
    W = np.asarray(W, dtype=np.float32)
    nc = _get_nc()
    in_maps = [host_prep(inputs, W, k) for k in range(NCORES)]
    res = run_bass_kernel_spmd(
        nc, in_maps, core_ids=list(range(NCORES)), trace=_trace
    )
    kernel.last_results = res
    return postprocess([res.results[k]["out"] for k in range(NCORES)])


if __name__ == "__main__":
    rng = np.random.default_rng(0)
    x = rng.normal(size=(B, N, Di)).astype(np.float32)
    w = (rng.normal(size=(C, N, Dc, Di)) / np.sqrt(Di)).astype(np.float32)
    out = kernel(x, w)
    print("out", out.shape, out.dtype, np.abs(out).max())
